# revision 8
# baseline (speedup 1.0000x reference)
"""Trainium2 Bass kernel for a 3-layer GCN (nn_BaselineGCN).

Strategy (8 NeuronCores, dst-partitioned nodes, aggregate-first layers):
  - Host: compute deg/dis; table row numbering is chunk-major (windows are
    grouped into AG chunks, rows ordered [chunk][core][window][p]) so the
    runtime AllGather can be emitted incrementally in contiguous slices.
    Edges (+ explicit self loops) are bucketed per (dst-window, src-row
    parity); gather indices are row>>1 (int16) with elem_step = 2 rows so a
    single table serves both parities.
  - Layer 1 aggregates directly over T0 = dis*x, which is host-precomputed
    and replicated to every core as an input: no AllGather and no matmul
    phase before the first gather -- edge processing starts immediately.
  - Per window: dma_gather source rows, one-hot(dst_local) on DVE,
    segment-sum via PE matmul accumulation in PSUM (the self term is one
    extra identity matmul against the SBUF-resident own-chunk tile).
  - Epilogue (L1/L2): s = dis*(agg), transpose via PE, s^T @ W -> relu ->
    next table tile T_{l+1} = dis*H, kept in SBUF + written to a local
    stage; AllGather chunks fire as their windows complete so the
    collective hides under the remaining gather work. L2's epilogue also
    computes Z3 = dis*(H2@W3) (64 cols padded in a 128-wide table) so L3
    aggregates in the narrow output space.
  - Layer 3: aggregate Z3, add self, scale by dis, write out.
"""
import os
import sys

sys.path.insert(0, "/opt/trn_rl_repo")

import numpy as np

NC_CORES = 8
GMAX = 8   # max groups (=1024 indices) per dma_gather call
GATH_BUFS = 4  # gather-tile pool depth (first GATH_BUFS windows are memset)


def _cdiv(a, b):
    return (a + b - 1) // b


# ---------------------------------------------------------------------------
# Host-side preprocessing
# ---------------------------------------------------------------------------
def make_chunks(NWIN):
    """Window ranges for incremental AllGather; last chunk small to minimize
    the exposed tail at the layer boundary."""
    if NWIN >= 10:
        bs = [int(NWIN * 0.40), int(NWIN * 0.72), int(NWIN * 0.92), NWIN]
    elif NWIN >= 4:
        bs = [NWIN // 2, NWIN - 1, NWIN]
    else:
        bs = [NWIN]
    bounds, prev = [], 0
    for b in bs:
        if b > prev:
            bounds.append((prev, b))
            prev = b
    return bounds  # list of (a, b) window ranges


def preprocess(edge_index, N):
    src = np.asarray(edge_index[0], dtype=np.int64)
    dst = np.asarray(edge_index[1], dtype=np.int64)
    deg = np.bincount(dst, minlength=N).astype(np.float32) + np.float32(1.0)
    dis = (np.float32(1.0) / np.sqrt(deg)).astype(np.float32)

    CH = N // NC_CORES
    NWIN = _cdiv(CH, 128)
    CHP = NWIN * 128
    NT = NC_CORES * CHP
    chunks = make_chunks(NWIN)   # [(a, b)]
    # chunk-major table row map: row(node) for nodes 0..N-1
    chunk_of_w = np.zeros(NWIN, np.int64)
    chunk_a = np.zeros(NWIN, np.int64)
    chunk_rows = np.zeros(NWIN, np.int64)   # rows per core in window's chunk
    chunk_base = np.zeros(NWIN, np.int64)
    for k, (a, b) in enumerate(chunks):
        chunk_of_w[a:b] = k
        chunk_a[a:b] = a
        chunk_rows[a:b] = (b - a) * 128
        chunk_base[a:b] = NC_CORES * 128 * a
    nodes = np.arange(N, dtype=np.int64)
    nc_ = nodes // CH
    no_ = nodes % CH
    nw_ = no_ >> 7
    np_ = no_ & 127
    rowmap = (chunk_base[nw_] + nc_ * chunk_rows[nw_]
              + (nw_ - chunk_a[nw_]) * 128 + np_)

    # self loops are NOT added to the edge list: the device adds the self
    # term via one identity matmul per window against the own-chunk tile
    src_a, dst_a = src, dst
    srow = rowmap[src_a]
    par = (srow & 1).astype(np.int64)
    idx16 = (srow >> 1).astype(np.int64)
    assert idx16.max() < 32768
    d_core = dst_a // CH
    d_loc = dst_a % CH
    d_w = d_loc >> 7
    d_l = d_loc & 127

    counts = np.zeros((NC_CORES, NWIN, 2), dtype=np.int64)
    percore = []
    for c in range(NC_CORES):
        sel = d_core == c
        ei, ed, w, h = idx16[sel], d_l[sel], d_w[sel], par[sel]
        order = np.lexsort((ed, h, w))
        ei, ed, w, h = ei[order], ed[order], w[order], h[order]
        np.add.at(counts[c], (w, h), 1)
        percore.append((ei, ed, w, h))

    G = _cdiv(counts, 128).max(axis=0)  # [NWIN, 2]
    assert (G.sum(axis=1) > 0).all(), "empty window unsupported"

    import ml_dtypes

    # shared call schedule: per (window, parity), gather calls of <=GMAX groups
    calls = []  # (wi, hi, g0, gc)
    for wi in range(NWIN):
        for hi in range(2):
            g0 = 0
            while g0 < G[wi, hi]:
                gc = min(GMAX, G[wi, hi] - g0)
                calls.append((wi, hi, g0, gc))
                g0 += gc

    cores = []
    for c in range(NC_CORES):
        ei, ed, w, h = percore[c]
        idx_parts, dstl_parts = [], []
        pos = 0
        for wi in range(NWIN):
            for hi in range(2):
                n = counts[c, wi, hi]
                g = G[wi, hi]
                seg_idx = np.full(g * 128, -1, dtype=np.int16)
                seg_dstl = np.full(g * 128, 255.0, dtype=np.float32)
                if n:
                    seg_idx[:n] = ei[pos:pos + n].astype(np.int16)
                    seg_dstl[:n] = ed[pos:pos + n].astype(np.float32)
                    pos += n
                idx_parts.append(seg_idx)
                dstl_parts.append(seg_dstl)
        idx_all = np.concatenate(idx_parts)
        dstl_all = np.concatenate(dstl_parts)
        TOT_G = len(idx_all) // 128

        # per-call valid counts; empty calls keep one dummy idx (0) because a
        # zero-valid gather is undefined
        seg_base = {}
        pos2 = 0
        for wi in range(NWIN):
            for hi in range(2):
                seg_base[(wi, hi)] = pos2
                pos2 += G[wi, hi] * 128
        ncounts = np.zeros(len(calls), dtype=np.int32)
        for k, (wi, hi, g0, gc) in enumerate(calls):
            n = int(counts[c, wi, hi])
            v = min(max(n - g0 * 128, 0), gc * 128)
            if v == 0:
                idx_all[seg_base[(wi, hi)] + g0 * 128] = 0
                v = 1
            ncounts[k] = v

        # device layouts
        idx_tiled = np.tile(idx_all.reshape(-1, 16).T, (8, 1)).copy()
        dstl_tiled = np.ascontiguousarray(
            dstl_all.reshape(TOT_G, 128).T).astype(ml_dtypes.bfloat16)
        d = np.ones(CHP, np.float32)
        d[:CH] = dis[c * CH:(c + 1) * CH]
        dis_win = np.ascontiguousarray(d.reshape(NWIN, 128).T)
        cores.append(dict(idx=idx_tiled, dstl=dstl_tiled, dis_win=dis_win,
                          ncounts=ncounts))
    return dis, rowmap, G, cores, CH, NWIN, CHP, NT, chunks, len(calls)


# ---------------------------------------------------------------------------
# Bass program
# ---------------------------------------------------------------------------
def build_program(DIN, DH, DOUT, G, NWIN, CHP, NT, chunks, TOT_IDX, TOT_G,
                  G_CAP, NCALLS, biases_nonzero):
    from concourse import bacc, bass, tile, mybir

    f32 = mybir.dt.float32
    bf16 = mybir.dt.bfloat16
    i16 = mybir.dt.int16
    ADD = mybir.AluOpType.add
    EQ = mybir.AluOpType.is_equal
    CPY = mybir.ActivationFunctionType.Copy
    RELU = mybir.ActivationFunctionType.Relu
    DZ = 128  # padded width of the Z3 table

    nc = bacc.Bacc("TRN2", target_bir_lowering=False, debug=False,
                   enable_asserts=False, num_devices=NC_CORES,
                   num_swdge_queues=4, dynamic_dma_scratch_size=32768)

    # --- I/O tensors ---
    t0_d = nc.dram_tensor("T0", [NT, DIN], bf16, kind="ExternalInput")
    W_d = [nc.dram_tensor("W0", [DIN, DH], bf16, kind="ExternalInput"),
           nc.dram_tensor("W1", [DH, DH], bf16, kind="ExternalInput"),
           nc.dram_tensor("W2", [DH, DOUT], bf16, kind="ExternalInput")]
    bias_d = [nc.dram_tensor(f"bias{i}", [128, d], f32, kind="ExternalInput")
              if biases_nonzero[i] else None
              for i, d in enumerate([DH, DH, DOUT])]
    idx_d = nc.dram_tensor("idx", [128, TOT_IDX // 16], i16,
                           kind="ExternalInput")
    dstl_d = nc.dram_tensor("dstl", [128, TOT_G], bf16, kind="ExternalInput")
    iotag_d = nc.dram_tensor("iotag", [128, 128 * G_CAP], bf16,
                             kind="ExternalInput")
    dis_d = nc.dram_tensor("dis_win", [128, NWIN], f32, kind="ExternalInput")
    ident_d = nc.dram_tensor("ident", [128, 128], bf16, kind="ExternalInput")
    cnt_d = nc.dram_tensor("ncounts", [1, NCALLS], mybir.dt.int32,
                           kind="ExternalInput")
    out_d = nc.dram_tensor("out", [CHP, DOUT], f32, kind="ExternalOutput")

    with tile.TileContext(nc) as tc:
        with (
            tc.tile_pool(name="const", bufs=1) as constp,
            tc.tile_pool(name="own", bufs=1) as ownp,
            tc.tile_pool(name="gath", bufs=GATH_BUFS) as gathp,
            tc.tile_pool(name="oh", bufs=4) as ohp,
            tc.tile_pool(name="epi", bufs=3) as epip,
            tc.tile_pool(name="psw", bufs=2, space="PSUM") as pswp,
            tc.tile_pool(name="pst", bufs=2, space="PSUM") as pstp,
            tc.tile_pool(name="ps2", bufs=2, space="PSUM") as ps2p,
            tc.tile_pool(name="ps3", bufs=2, space="PSUM") as ps3p,
            tc.tile_pool(name="dram", bufs=1, space="DRAM") as dramp,
        ):
            # --- persistent SBUF constants ---
            idx_t = constp.tile([128, TOT_IDX // 16], i16, tag="idx")
            nc.sync.dma_start(idx_t[:], idx_d[:])
            dstl_t = constp.tile([128, TOT_G], bf16, tag="dstl")
            nc.sync.dma_start(dstl_t[:], dstl_d[:])
            iotag_t = constp.tile([128, 128 * G_CAP], bf16, tag="iotag")
            nc.sync.dma_start(iotag_t[:], iotag_d[:])
            dis_t = constp.tile([128, NWIN], f32, tag="dis")
            nc.sync.dma_start(dis_t[:], dis_d[:])
            ident_t = constp.tile([128, 128], bf16, tag="ident")
            nc.sync.dma_start(ident_t[:], ident_d[:])
            cnt_t = constp.tile([1, NCALLS], mybir.dt.int32, tag="cnt")
            nc.sync.dma_start(cnt_t[:], cnt_d[:])
            cnt_regs = [nc.gpsimd.alloc_register(f"gcnt{i}") for i in range(4)]
            bias_t = []
            for i, d in enumerate([DH, DH, DOUT]):
                if biases_nonzero[i]:
                    bt = constp.tile([128, d], f32, tag=f"bias{i}")
                    nc.sync.dma_start(bt[:], bias_d[i][:])
                    bias_t.append(bt)
                else:
                    bias_t.append(None)
            # weight tiles (lhs contraction split in 128-row tiles)
            wk = []
            for li, (kd, od) in enumerate([(DIN, DH), (DH, DH), (DH, DOUT)]):
                wkl = []
                for k in range(kd // 128):
                    wt_ = constp.tile([128, od], bf16, tag=f"wk{li}_{k}")
                    nc.sync.dma_start(wt_[:], W_d[li][k * 128:(k + 1) * 128, :])
                    wkl.append(wt_)
                wk.append(wkl)

            # --- own-chunk table tiles (self term + next-table stage) ---
            t0own = ownp.tile([128, NWIN, DIN], bf16, tag="t0own",
                              name="t0own")
            t1own = ownp.tile([128, NWIN, DH], bf16, tag="t1own",
                              name="t1own")
            z3own = ownp.tile([128, NWIN, DZ], bf16, tag="z3own",
                              name="z3own")
            # cols DOUT..DZ are never written by the epilogue but flow into
            # the (unread) upper psum cols; keep them finite
            nc.vector.memset(z3own[:], 0.0)
            # load own chunk of T0 (per-chunk contiguous in the table);
            # rows for this core start at chunk_base + core*chunk_rows.
            # We express the core-dependent base via the per-core input
            # layout instead: host passes t0own rows as a separate input.
            t0own_d = nc.dram_tensor("t0own_in", [CHP, DIN], bf16,
                                     kind="ExternalInput")
            nc.sync.dma_start(
                t0own[:],
                t0own_d[:].rearrange("(w p) e -> p w e", p=128))

            # --- DRAM tables and stages ---
            t1full = dramp.tile([NT, DH], bf16, tag="t1full",
                                name="t1full")
            z3full = dramp.tile([NT, DZ], bf16, tag="z3full",
                                name="z3full")
            t1stage = dramp.tile([CHP, DH], bf16, tag="t1stage",
                                 name="t1stage")
            z3stage = dramp.tile([CHP, DZ], bf16, tag="z3stage",
                                 name="z3stage")

            RG = [list(range(NC_CORES))]

            def emit_ag(src_stage, dst_full, a, b, width):
                rows = (b - a) * 128
                base = NC_CORES * 128 * a
                nc.gpsimd.collective_compute(
                    "AllGather", bass.mybir.AluOpType.bypass,
                    replica_groups=RG,
                    ins=[src_stage[a * 128:b * 128, :]],
                    outs=[dst_full[base:base + NC_CORES * rows, :].opt()])

            tabs = [(t0_d, DIN, t0own), (t1full, DH, t1own),
                    (z3full, DZ, z3own)]

            for li in range(3):
                tab_d, Dt, own_t = tabs[li]
                idx_off16 = 0
                g_off = 0
                qrr = 0
                call_i = 0
                for w in range(NWIN):
                    Gl, Gh = int(G[w, 0]), int(G[w, 1])
                    Gt = Gl + Gh
                    wt = gathp.tile([128, G_CAP, Dt], bf16, tag="gather",
                                    name="wt")
                    # first GATH_BUFS tiles of each distinct shape class:
                    # clear so slots skipped by short gathers never hold NaN
                    # bit patterns (li 0/1 share a shape; li 2 is narrower)
                    nalloc = (li * NWIN + w) if li < 2 else w
                    if nalloc < GATH_BUFS:
                        nc.vector.memset(wt[:], 0.0)
                    for parity, gcnt, gbase in ((0, Gl, 0), (1, Gh, Gl)):
                        tbl = tab_d[parity::2, :]
                        g0 = 0
                        while g0 < gcnt:
                            gc = min(GMAX, gcnt - g0)
                            reg = cnt_regs[qrr % 4]
                            nc.gpsimd.reg_load(
                                reg, cnt_t[0:1, call_i:call_i + 1])
                            nc.gpsimd.dma_gather(
                                wt[:, gbase + g0:gbase + g0 + gc, :],
                                tbl,
                                idx_t[:, idx_off16:idx_off16 + gc * 8],
                                num_idxs=gc * 128,
                                num_idxs_reg=reg,
                                elem_size=Dt,
                                elem_step=2 * Dt,
                                queue_num=qrr % 4,
                            )
                            qrr += 1
                            call_i += 1
                            idx_off16 += gc * 8
                            g0 += gc
                    # one-hot: oh[p, j, g] = (dstl[p, g] == j)
                    oh = ohp.tile([128, 128, G_CAP], bf16, tag="oh",
                                  name="oh")
                    nc.vector.tensor_tensor(
                        oh[:, :, :Gt],
                        dstl_t[:, g_off:g_off + Gt].unsqueeze(1)
                            .broadcast_to((128, 128, Gt)),
                        iotag_t[:].rearrange("p (j g) -> p j g", g=G_CAP)
                            [:, :, :Gt],
                        op=EQ,
                    )
                    g_off += Gt
                    # segment-sum via PE; self term = identity matmul of the
                    # own-chunk window tile
                    psw = pswp.tile([128, Dt], f32, tag="psw", name="psw")
                    for g in range(Gt):
                        nc.tensor.matmul(psw[:], oh[:, :, g], wt[:, g, :],
                                         start=(g == 0), stop=False)
                    nc.tensor.matmul(psw[:], ident_t[:], own_t[:, w, :],
                                     start=False, stop=True)

                    dis_w = dis_t[:, w:w + 1]
                    if li < 2:
                        # s = dis * agg (bf16), transpose, s^T @ W
                        s = epip.tile([128, DH], bf16, tag="s", name="s")
                        nc.scalar.activation(s[:], psw[:], CPY, scale=dis_w)
                        sT = epip.tile([128, DH], bf16, tag="sT", name="sT")
                        for k in range(DH // 128):
                            pst = pstp.tile([128, 128], bf16, tag="pst",
                                            name="pst")
                            nc.tensor.transpose(
                                pst[:], s[:, k * 128:(k + 1) * 128],
                                ident_t[:])
                            nc.vector.tensor_copy(
                                sT[:, k * 128:(k + 1) * 128], pst[:])
                        ps2 = ps2p.tile([128, DH], f32, tag="ps2", name="ps2")
                        KT = DH // 128
                        for k in range(KT):
                            nc.tensor.matmul(
                                ps2[:], sT[:, k * 128:(k + 1) * 128],
                                wk[li][k][:],
                                start=(k == 0), stop=(k == KT - 1))
                        pre = ps2
                        if bias_t[li] is not None:
                            tb = epip.tile([128, DH], f32, tag="tb",
                                           name="tb")
                            nc.vector.tensor_tensor(tb[:], ps2[:],
                                                    bias_t[li][:], op=ADD)
                            pre = tb
                        if li == 0:
                            # T1 = dis * relu(pre)
                            nc.scalar.activation(t1own[:, w, :], pre[:],
                                                 RELU, scale=dis_w)
                            nc.sync.dma_start(
                                t1stage[w * 128:(w + 1) * 128, :],
                                t1own[:, w, :])
                        else:
                            # H2 then Z3 = dis * (H2 @ W3)
                            h2 = epip.tile([128, DH], bf16, tag="h2",
                                           name="h2")
                            nc.scalar.activation(h2[:], pre[:], RELU)
                            h2T = epip.tile([128, DH], bf16, tag="h2T",
                                            name="h2T")
                            for k in range(DH // 128):
                                pst = pstp.tile([128, 128], bf16, tag="pst",
                                                name="pst")
                                nc.tensor.transpose(
                                    pst[:], h2[:, k * 128:(k + 1) * 128],
                                    ident_t[:])
                                nc.vector.tensor_copy(
                                    h2T[:, k * 128:(k + 1) * 128], pst[:])
                            ps3 = ps3p.tile([128, DOUT], f32, tag="ps3",
                                            name="ps3")
                            for k in range(KT):
                                nc.tensor.matmul(
                                    ps3[:], h2T[:, k * 128:(k + 1) * 128],
                                    wk[2][k][:],
                                    start=(k == 0), stop=(k == KT - 1))
                            nc.scalar.activation(z3own[:, w, :DOUT], ps3[:],
                                                 CPY, scale=dis_w)
                            nc.sync.dma_start(
                                z3stage[w * 128:(w + 1) * 128, :],
                                z3own[:, w, :])
                        # incremental AllGather of the next-layer table
                        for (a, b) in chunks:
                            if w == b - 1:
                                if li == 0:
                                    emit_ag(t1stage, t1full, a, b, DH)
                                else:
                                    emit_ag(z3stage, z3full, a, b, DZ)
                    else:
                        # final layer: out = dis * (agg + self) [+ b3]
                        o = epip.tile([128, DOUT], f32, tag="o", name="o")
                        nc.scalar.activation(o[:], psw[:, :DOUT], CPY,
                                             scale=dis_w)
                        if bias_t[2] is not None:
                            nc.vector.tensor_tensor(o[:], o[:], bias_t[2][:],
                                                    op=ADD)
                        nc.sync.dma_start(out_d[w * 128:(w + 1) * 128, :],
                                          o[:])
    nc.compile()
    return nc


# ---------------------------------------------------------------------------
# Entry point
# ---------------------------------------------------------------------------
def kernel(x, edge_index, W1, b1, W2, b2, W3, b3):
    from concourse.bass_utils import run_bass_kernel_spmd
    import ml_dtypes

    bfnp = ml_dtypes.bfloat16
    x = np.asarray(x, dtype=np.float32)
    Ws = [np.asarray(w, dtype=np.float32) for w in (W1, W2, W3)]
    bs = [np.asarray(b, dtype=np.float32) for b in (b1, b2, b3)]

    N, DIN = x.shape
    DH = Ws[0].shape[1]
    DOUT = Ws[2].shape[1]

    (dis, rowmap, G, cores, CH, NWIN, CHP, NT, chunks,
     NCALLS) = preprocess(edge_index, N)
    TOT_IDX = cores[0]["idx"].shape[1] * 16
    TOT_G = cores[0]["dstl"].shape[1]
    G_CAP = int((G[:, 0] + G[:, 1]).max())
    biases_nonzero = [bool(np.any(b != 0)) for b in bs]

    nc = build_program(DIN, DH, DOUT, G, NWIN, CHP, NT, chunks, TOT_IDX,
                       TOT_G, G_CAP, NCALLS, biases_nonzero)

    # host-side tables
    T0 = np.zeros((NT, DIN), dtype=bfnp)
    T0[rowmap] = (dis[:, None] * x).astype(bfnp)
    ident = np.eye(128, dtype=bfnp)
    iotag = np.tile(np.repeat(np.arange(128), G_CAP).astype(bfnp), (128, 1))

    in_maps = []
    for c in range(NC_CORES):
        # own chunk rows of T0 in window order (w*128+p)
        own_rows = np.zeros((CHP, DIN), dtype=bfnp)
        base = c * CH
        own_rows[:CH] = T0[rowmap[base:base + CH]]
        m = {
            "T0": T0,
            "t0own_in": own_rows,
            "idx": cores[c]["idx"],
            "dstl": cores[c]["dstl"],
            "iotag": iotag,
            "dis_win": cores[c]["dis_win"],
            "ident": ident,
            "ncounts": cores[c]["ncounts"][None, :],
            "W0": Ws[0].astype(bfnp),
            "W1": Ws[1].astype(bfnp),
            "W2": Ws[2].astype(bfnp),
        }
        for i in range(3):
            if biases_nonzero[i]:
                m[f"bias{i}"] = np.tile(bs[i][None, :], (128, 1))
        in_maps.append(m)

    trace = bool(int(os.environ.get("GCN_TRACE", "0")))
    res = run_bass_kernel_spmd(nc, in_maps, core_ids=list(range(NC_CORES)),
                               trace=trace)
    kernel.last_results = res
    out = np.concatenate([res.results[c]["out"][:CH]
                          for c in range(NC_CORES)], axis=0)
    return out.astype(np.float32)
